# revision 71
# baseline (speedup 1.0000x reference)
"""Trainium2 Bass kernel for nn_BinaryLabelSoftRouter.

Reference computation (B=16, T=1024, D=2048, H=256, H2=128):
  base   = where(labels>0, [.25,.75], [.75,.25])            # (B,T,2)
  h1     = gelu(LN(x @ W1 + b1) * g1 + be1)                 # erf gelu
  h2     = gelu(LN(h1 @ W2 + b2) * g2 + be2)
  adj    = tanh(h2 @ W3 + b3) * 0.1
  p      = softmax((base + adj) / clip(temp, .1), -1)       # (B,T,2)
  out    = EMA over T (s_t = .9 s_{t-1} + .1 p_t, s_0 = p_0)

Sharding: data-parallel over batch, 2 rows per core x 8 cores.

Device-side rewrites (all exact up to fp rounding):
  * softmax over 2 classes -> sigmoid of the logit difference.
  * EMA over each 128-step chunk is a lower-triangular [128,128] matmul
    plus a carry matmul from the previous chunk's last element.
  * gelu via erf:  2*gelu(x) = x*(1+erf(x/sqrt(2))).  The factor 2 on
    h1g cancels inside LN2 when LN2's eps is scaled 4x; the factor 2 on
    h2g is folded into W3 (host-side W3/2).  This keeps the scalar
    engine inside ONE activation-table set (sigmoid_and_others: copy /
    erf / sigmoid / tanh) -- act-table swaps cost ~1.3us each.
  * rstd = 1/sqrt(var+eps) via fast-inverse-sqrt (magic constant + 2
    Newton steps) on the vector engine, batched over 4 chunks, because
    Sqrt lives in a different act-table set.  The Newton iteration is
    signed so the final rstd comes out negative; the host passes -g1/-g2
    so the product is exact.

Main matmuls run in bf16 (fp32 PSUM accumulation) -> end-to-end rel
error vs the fp32 reference ~1e-4.  X is cast fp32->bf16 during the
HBM->SBUF DMA (SWDGE cast) and transposed on the tensor engine.
EMA matmuls run in fp32.
"""

import os
import numpy as np
import ml_dtypes

B, T, AD = 16, 1024, 2048
HID1, HID2 = 256, 128
NCORES = 8
B_LOC = B // NCORES            # 2 rows per core
CH_ROW = T // 128              # 8 chunks per row
CH = B_LOC * CH_ROW            # 16 chunks per core
GRP = 4                        # chunks per LN/head batch group
KC = AD // 128                 # 16 contraction chunks for mm1
SM = 0.9
ADJ = 0.1
LN_EPS = 1e-5
MAGIC = 0x5f3759df - 0x00400000   # seed for rsqrt of v2 = v/2

_BF16 = ml_dtypes.bfloat16

_NC = {}
LAST_RESULTS = None


def _make_ema_mats():
    """EMA-as-matmul constants, all pre-transposed to lhsT layout [k, tau].

    s_c = A_loc @ p_c + 0.9^(tau+1) * s_{c-1}[127] and the carry expands
    into rank-1 matmuls against p_{c-1}, p_{c-2}: contributions beyond
    depth 2 carry a 0.9^256 ~ 1.8e-12 factor -> exactly zero in fp32.
    This removes the serial cross-chunk dependency entirely.
    """
    tau = np.arange(128, dtype=np.float64)
    diff = tau[:, None] - tau[None, :]
    Am = np.where(diff >= 0, 0.1 * SM ** diff, 0.0)
    A0 = Am.copy()
    A0[:, 0] = SM ** tau
    dec = SM ** (tau + 1.0)          # 0.9^(tau+1)
    r1f = np.outer(A0[127, :], dec)  # [k, tau], carry from chunk 0
    r1m = np.outer(Am[127, :], dec)
    r2f = (SM ** 128) * r1f
    r2m = (SM ** 128) * r1m
    f32c = lambda a: np.ascontiguousarray(a, np.float32)
    return {
        "a0t": f32c(A0.T), "amt": f32c(Am.T),
        "r1f": f32c(r1f), "r1m": f32c(r1m),
        "r2f": f32c(r2f), "r2m": f32c(r2m),
    }


def _build_nc(sim_gelu=False, triv1=True, triv2=True, trivb3=True):
    # trivN: layer-N has b==0, g==1, be==0 (true for this problem's
    # setup_inputs); skips the bias matmul and the affine stt ops.
    # trivb3: b3 == 0.
    # sim_gelu: CoreSim has no Erf LUT; substitute Tanh so the identical
    # program structure can run under the simulator (race/OOB checks).
    import concourse.mybir as mybir
    import concourse.tile as tile
    from concourse import bacc

    f32 = mybir.dt.float32
    bf16 = mybir.dt.bfloat16
    i32 = mybir.dt.int32
    AF = mybir.ActivationFunctionType
    OP = mybir.AluOpType
    ERF = AF.Tanh if sim_gelu else AF.Erf
    INV_SQRT2 = float(1.0 / np.sqrt(2.0))

    nc = bacc.Bacc()

    # ---- DRAM parameters (per-core) ----
    x_d = nc.declare_dram_parameter("x", [B_LOC, T, AD], f32, isOutput=False)
    lab_d = nc.declare_dram_parameter("labels", [CH, 128], i32, isOutput=False)
    w1_d = nc.declare_dram_parameter("w1", [128, KC, HID1], bf16, isOutput=False)
    w2_d = nc.declare_dram_parameter("w2", [128, 2, HID2], bf16, isOutput=False)
    w3_d = nc.declare_dram_parameter("w3", [128, 2], bf16, isOutput=False)
    b1_d = nc.declare_dram_parameter("b1", [1, HID1], bf16, isOutput=False)
    b2_d = nc.declare_dram_parameter("b2", [1, HID2], bf16, isOutput=False)
    b3_d = nc.declare_dram_parameter("b3g", [128, 2 * GRP], f32, isOutput=False)
    g1_d = nc.declare_dram_parameter("g1bn", [128, HID1], f32, isOutput=False)
    be1_d = nc.declare_dram_parameter("be1b", [128, HID1], f32, isOutput=False)
    g2_d = nc.declare_dram_parameter("g2bn", [128, HID2], f32, isOutput=False)
    be2_d = nc.declare_dram_parameter("be2b", [128, HID2], f32, isOutput=False)
    ema_d = {
        name: nc.declare_dram_parameter(name, [128, 128], f32, isOutput=False)
        for name in ("a0t", "amt", "r1f", "r1m", "r2f", "r2m")
    }
    idb_d = nc.declare_dram_parameter("idbf", [128, 128], bf16, isOutput=False)
    idf_d = nc.declare_dram_parameter("idf32", [16, 16], f32, isOutput=False)
    ones_d = nc.declare_dram_parameter("ones1", [1, 128], bf16, isOutput=False)
    magic_d = nc.declare_dram_parameter("magici", [128, 1], i32, isOutput=False)
    it_d = nc.declare_dram_parameter("itb", [128, 1], f32, isOutput=False)
    nit_d = nc.declare_dram_parameter("nitb", [128, 1], f32, isOutput=False)
    out_d = nc.declare_dram_parameter("out", [B_LOC, T, 2], f32, isOutput=True)

    with tile.TileContext(nc) as tc:
        with (
            tc.tile_pool(name="singles", bufs=1) as singles,
            tc.tile_pool(name="xpool", bufs=3) as xpool,
            tc.tile_pool(name="xtpool", bufs=2) as xtpool,
            tc.tile_pool(name="act", bufs=4) as act,
            tc.tile_pool(name="hbuf", bufs=10) as hbuf,
            tc.tile_pool(name="stat", bufs=6) as stat,
            tc.tile_pool(name="ptp", bufs=3, space="PSUM") as ptp,
            tc.tile_pool(name="ptph", bufs=1, space="PSUM") as ptph,
            tc.tile_pool(name="pmm", bufs=2, space="PSUM") as pmm,
            tc.tile_pool(name="py", bufs=1, space="PSUM") as py,
            tc.tile_pool(name="ps", bufs=1, space="PSUM") as ps,
        ):
            # ---- resident tiles ----
            def load(name, shape, dt, src):
                t = singles.tile(shape, dt, tag=name)
                nc.sync.dma_start(t[:], src[:])
                return t

            # critical-path loads only; the rest is deferred until after
            # the first group's front end is emitted, so the first chunk's
            # activations aren't queued behind ~2 MB of constants.
            # chunk 0 goes via HWDGE fp32 + on-chip cast: it skips the
            # gpsimd SWDGE preamble and heads the DMA queue, so the tensor
            # engine starts transposing several us earlier.
            xc0f = singles.tile([128, AD], f32, tag="xc0f")
            nc.sync.dma_start(xc0f[:], x_d[0, 0:128, :])
            idb_s = load("idb", [128, 128], bf16, idb_d)
            w1_s = load("w1", [128, KC, HID1], bf16, w1_d)
            ones_s = (None if (triv1 and triv2)
                      else load("ones", [1, 128], bf16, ones_d))
            b1_s = None if triv1 else load("b1", [1, HID1], bf16, b1_d)
            idf_s = load("idf", [16, 16], f32, idf_d)

            # label prep: Lh[tau, chunk] = labels - 0.5 (tiny; done first
            # so its PE transpose doesn't stall the stream mid-kernel)
            lab_i = singles.tile([CH, 128], i32)
            nc.sync.dma_start(lab_i[:], lab_d[:])
            lab_f = singles.tile([CH, 128], f32)
            nc.vector.tensor_copy(lab_f[:], lab_i[:])
            p_lab = py.tile([128, CH], f32, tag="y")
            nc.tensor.transpose(p_lab[:], lab_f[:], idf_s[:])
            lh_s = singles.tile([128, CH], f32)
            nc.vector.tensor_scalar(
                out=lh_s[:], in0=p_lab[:], scalar1=0.5, scalar2=None,
                op0=OP.subtract)

            def load_rest():
                nonlocal w2_s, w3_s, b2_s, b3g_s, g1_s, be1_s, g2_s, \
                    be2_s, ema_s, magic_s, it_s, nit_s
                w2_s = load("w2", [128, 2, HID2], bf16, w2_d)
                w3_s = load("w3", [128, 2], bf16, w3_d)
                b2_s = None if triv2 else load("b2", [1, HID2], bf16, b2_d)
                b3g_s = (None if trivb3
                         else load("b3g", [128, 2 * GRP], f32, b3_d))
                g1_s = be1_s = g2_s = be2_s = None
                if not triv1:
                    g1_s = load("g1", [128, HID1], f32, g1_d)  # holds -g1
                    be1_s = load("be1", [128, HID1], f32, be1_d)
                if not triv2:
                    g2_s = load("g2", [128, HID2], f32, g2_d)  # holds -g2
                    be2_s = load("be2", [128, HID2], f32, be2_d)
                ema_s = {name: load(name, [128, 128], f32, d)
                         for name, d in ema_d.items()}
                magic_s = load("magic", [128, 1], i32, magic_d)
                it_s = load("it", [128, 1], f32, it_d)
                nit_s = load("nit", [128, 1], f32, nit_d)

            w2_s = w3_s = b2_s = b3g_s = g1_s = be1_s = g2_s = be2_s = None
            ema_s = magic_s = it_s = nit_s = None

            s_all = singles.tile([128, CH, 2], f32)
            pc_full = singles.tile([128, CH, 2], f32)

            def rsqrt_full(var_ap, n, eps, tagsuf):
                """negative 1/sqrt(var+eps) batched over n columns (fast
                inverse sqrt + 2 Newton steps; the sign is folded into the
                negated gains -g1/-g2 on the host side)."""
                v2 = stat.tile([128, n], f32, tag="v2" + tagsuf)
                nc.vector.tensor_scalar(
                    out=v2[:], in0=var_ap, scalar1=0.5, scalar2=0.5 * eps,
                    op0=OP.mult, op1=OP.add)
                ib = stat.tile([128, n], i32, tag="ib" + tagsuf)
                nc.vector.tensor_scalar(
                    out=ib[:], in0=v2[:].bitcast(i32), scalar1=1,
                    scalar2=None, op0=OP.logical_shift_right)
                y = stat.tile([128, n], f32, tag="y" + tagsuf)
                nc.vector.tensor_tensor(
                    out=y[:].bitcast(i32),
                    in0=magic_s[:].to_broadcast((128, n)), in1=ib[:],
                    op=OP.subtract)          # y0 = +seed
                p = stat.tile([128, n], f32, tag="p" + tagsuf)
                # iter 1: y1 = y0*(1.5 - v2*y0^2)  -> computed as
                #   p = y0*y0; q = p*v2; y1 = (q - 1.5)*y0 * -1 folded:
                # keep standard signs: y1 = (1.5 - q)*y0 via two ops
                nc.vector.tensor_tensor(out=p[:], in0=y[:], in1=y[:],
                                        op=OP.mult)
                nc.vector.tensor_tensor(out=p[:], in0=p[:], in1=v2[:],
                                        op=OP.mult)
                # y1n = (p - 1.5) * y0   = -y1   (negative)
                nc.vector.scalar_tensor_tensor(
                    out=y[:], in0=p[:], scalar=1.5, in1=y[:],
                    op0=OP.subtract, op1=OP.mult)
                # iter 2 on negative y1n: y1n^2 = y1^2 (sign cancels)
                nc.vector.tensor_tensor(out=p[:], in0=y[:], in1=y[:],
                                        op=OP.mult)
                nc.vector.tensor_tensor(out=p[:], in0=p[:], in1=v2[:],
                                        op=OP.mult)
                # y2n = (1.5 - p) * y1n  (stays negative):
                #     = (p - 1.5) * (-y1n)... use (p-1.5)*y1n = +y2; we
                # want negative output, so: y2n = (p - 1.5) * y1n * ...
                # (p-1.5) < 0 and y1n < 0 -> product positive = +y2.
                # One more negate folds into -g as planned, so produce +y2
                # here and pass -g:  final = (x-mu)*(-g)*(+y2)... wrong
                # sign.  Instead produce -y2: (1.5-p)*y1n.  No reverse
                # subtract available, so negate p first into (1.5-p) via
                # scalar_tensor_tensor with scalar=-1:
                #   y2n = ((p * -1) + 1.5) * y1n
                nc.vector.tensor_scalar(
                    out=p[:], in0=p[:], scalar1=-1.0, scalar2=1.5,
                    op0=OP.mult, op1=OP.add)
                nc.vector.tensor_tensor(out=y[:], in0=p[:], in1=y[:],
                                        op=OP.mult)   # negative rstd
                return y

            mv1G, h1sD, rstd1G = {}, {}, {}
            mv2G, h2sD, rstd2G, yallG = {}, {}, {}, {}
            xcD = {}

            def s1_chunk(c):
                """load + transpose + mm1 + LN1 stats for one chunk."""
                g, j = divmod(c, GRP)
                if j == 0:
                    mv1G[g] = stat.tile([128, GRP, 2], f32, tag="mv1", name=f"mv1_{g}")
                mv1 = mv1G[g]
                r, cc = divmod(c, CH_ROW)

                xc = xpool.tile([128, AD], bf16, tag="xc")
                if c == 0:
                    nc.vector.tensor_copy(out=xc[:], in_=xc0f[:])
                else:
                    for hh in range(2):
                        nc.gpsimd.dma_start(
                            out=xc[:, hh * (AD // 2):(hh + 1) * (AD // 2)],
                            in_=x_d[r, 128 * cc:128 * (cc + 1),
                                    hh * (AD // 2):(hh + 1) * (AD // 2)])

                xt = xtpool.tile([128, KC, 128], bf16, tag="xt")
                for tg in range(4):
                    ptile = ptp.tile([128, 512], bf16, tag="tp")
                    for tj in range(4):
                        k = 4 * tg + tj
                        nc.tensor.transpose(
                            ptile[:, 128 * tj:128 * (tj + 1)],
                            xc[:, 128 * k:128 * (k + 1)],
                            idb_s[:])
                    if tg % 2 == 0:
                        nc.scalar.activation(
                            out=xt[:, 4 * tg:4 * (tg + 1), :],
                            in_=ptile[:], func=AF.Copy)
                    else:
                        nc.vector.tensor_copy(
                            out=xt[:, 4 * tg:4 * (tg + 1), :],
                            in_=ptile[:])

                ph1 = pmm.tile([128, HID1], f32, tag="mm")
                for k in range(KC):
                    nc.tensor.matmul(
                        ph1[:], xt[:, k, :], w1_s[:, k, :],
                        start=(k == 0), stop=(triv1 and k == KC - 1))
                if not triv1:
                    nc.tensor.matmul(
                        ph1[:], ones_s[:], b1_s[:], start=False, stop=True)

                st6 = stat.tile([128, 6], f32, tag="st6")
                nc.vector.bn_stats(st6[:], ph1[:])
                nc.vector.bn_aggr(mv1[:, j, :], st6[:])
                h1s = hbuf.tile([128, HID1], f32, tag="h1s")
                nc.scalar.activation(out=h1s[:], in_=ph1[:], func=AF.Copy)
                h1sD[c] = h1s

            def s2a_chunk(c):
                """LN1 apply -> mm2 -> LN2 stats for one chunk."""
                g, j = divmod(c, GRP)
                if j == 0:
                    rstd1G[g] = rsqrt_full(mv1G[g][:, :, 1], GRP, LN_EPS,
                                           "a")
                    mv2G[g] = stat.tile([128, GRP, 2], f32, tag="mv2", name=f"mv2_{g}")
                mv1, rstd1, mv2 = mv1G[g], rstd1G[g], mv2G[g]
                h1s = h1sD.pop(c)

                xn = act.tile([128, HID1], f32, tag="xn")
                if triv1:
                    # xn = (h1 - mu) * (-rstd) = -LN(h1): one 2x-mode
                    # tensor_scalar; the sign cancels in the odd-erf
                    # gelu identity below.
                    nc.vector.tensor_scalar(
                        out=xn[:], in0=h1s[:], scalar1=mv1[:, j, 0:1],
                        scalar2=rstd1[:, j:j + 1],
                        op0=OP.subtract, op1=OP.mult)
                    sgn = -1.0
                else:
                    nc.vector.scalar_tensor_tensor(
                        out=xn[:], in0=h1s[:], scalar=mv1[:, j, 0:1],
                        in1=g1_s[:], op0=OP.subtract, op1=OP.mult)
                    nc.vector.scalar_tensor_tensor(
                        out=xn[:], in0=xn[:], scalar=rstd1[:, j:j + 1],
                        in1=be1_s[:], op0=OP.mult, op1=OP.add)
                    sgn = 1.0
                ef = act.tile([128, HID1], f32, tag="ef")
                nc.scalar.activation(out=ef[:], in_=xn[:], func=ERF,
                                     scale=INV_SQRT2)
                h1g = act.tile([128, HID1], bf16, tag="h1g")
                # 2*gelu(z) = (erf(z/sqrt2) + sgn) * xn  with xn=sgn*z
                nc.vector.scalar_tensor_tensor(
                    out=h1g[:], in0=ef[:], scalar=sgn, in1=xn[:],
                    op0=OP.add, op1=OP.mult)

                pt1 = ptph.tile([128, 512], bf16, tag="tph")
                for k in range(2):
                    nc.tensor.transpose(
                        pt1[:, 128 * k:128 * (k + 1)],
                        h1g[:, 128 * k:128 * (k + 1)],
                        idb_s[:])
                h1t = act.tile([128, 2, 128], bf16, tag="h1t")
                nc.scalar.activation(
                    out=h1t[:], in_=pt1[:, :256], func=AF.Copy)

                ph2 = pmm.tile([128, HID1], f32, tag="mm")
                for k in range(2):
                    nc.tensor.matmul(
                        ph2[:, :HID2], h1t[:, k, :], w2_s[:, k, :],
                        start=(k == 0), stop=(triv2 and k == 1))
                if not triv2:
                    nc.tensor.matmul(
                        ph2[:, :HID2], ones_s[:], b2_s[:], start=False,
                        stop=True)

                st6b = stat.tile([128, 6], f32, tag="st6")
                nc.vector.bn_stats(st6b[:], ph2[:, :HID2])
                nc.vector.bn_aggr(mv2[:, j, :], st6b[:])
                h2s = hbuf.tile([128, HID2], f32, tag="h2s")
                nc.scalar.activation(out=h2s[:], in_=ph2[:, :HID2],
                                     func=AF.Copy)
                h2sD[c] = h2s

            def s2b_chunk(c):
                """LN2 apply -> mm3 -> y for one chunk."""
                g, j = divmod(c, GRP)
                if j == 0:
                    # LN2 eps is 4x because h1g carries the factor 2
                    rstd2G[g] = rsqrt_full(mv2G[g][:, :, 1], GRP,
                                           4.0 * LN_EPS, "b")
                    yallG[g] = stat.tile([128, GRP, 2], f32, tag="yall",
                                         name=f"yall_{g}")
                mv2, rstd2, y_all = mv2G[g], rstd2G[g], yallG[g]
                h2s = h2sD.pop(c)

                xn2 = act.tile([128, HID2], f32, tag="xn2")
                if triv2:
                    nc.vector.tensor_scalar(
                        out=xn2[:], in0=h2s[:], scalar1=mv2[:, j, 0:1],
                        scalar2=rstd2[:, j:j + 1],
                        op0=OP.subtract, op1=OP.mult)
                    sgn2 = -1.0
                else:
                    nc.vector.scalar_tensor_tensor(
                        out=xn2[:], in0=h2s[:], scalar=mv2[:, j, 0:1],
                        in1=g2_s[:], op0=OP.subtract, op1=OP.mult)
                    nc.vector.scalar_tensor_tensor(
                        out=xn2[:], in0=xn2[:], scalar=rstd2[:, j:j + 1],
                        in1=be2_s[:], op0=OP.mult, op1=OP.add)
                    sgn2 = 1.0
                ef2 = act.tile([128, HID2], f32, tag="ef2")
                nc.scalar.activation(out=ef2[:], in_=xn2[:], func=ERF,
                                     scale=INV_SQRT2)
                h2g = act.tile([128, HID2], bf16, tag="h2g")
                nc.vector.scalar_tensor_tensor(
                    out=h2g[:], in0=ef2[:], scalar=sgn2, in1=xn2[:],
                    op0=OP.add, op1=OP.mult)

                pt2 = ptph.tile([128, 512], bf16, tag="tph")
                nc.tensor.transpose(pt2[:, :128], h2g[:], idb_s[:])
                h2t = act.tile([128, 128], bf16, tag="h2t")
                nc.vector.tensor_copy(out=h2t[:], in_=pt2[:, :128])
                pyt = py.tile([128, CH], f32, tag="y")
                nc.tensor.matmul(pyt[:, :2], h2t[:], w3_s[:],
                                 start=True, stop=True)
                nc.vector.tensor_copy(out=y_all[:, j, :], in_=pyt[:, :2])

            def head_ema(g):
                """batched head + EMA matmuls for one group."""
                y_all = yallG.pop(g)
                if not trivb3:
                    nc.vector.tensor_tensor(
                        out=y_all[:].rearrange("p g n -> p (g n)"),
                        in0=y_all[:].rearrange("p g n -> p (g n)"),
                        in1=b3g_s[:], op=OP.add)
                th = stat.tile([128, GRP, 2], f32, tag="th")
                nc.scalar.activation(
                    out=th[:].rearrange("p g n -> p (g n)"),
                    in_=y_all[:].rearrange("p g n -> p (g n)"),
                    func=AF.Tanh)
                dcol = stat.tile([128, GRP], f32, tag="dcol")
                nc.vector.tensor_tensor(
                    out=dcol[:], in0=th[:, :, 1], in1=th[:, :, 0],
                    op=OP.subtract)
                nc.vector.scalar_tensor_tensor(
                    out=dcol[:], in0=dcol[:], scalar=ADJ,
                    in1=lh_s[:, GRP * g:GRP * (g + 1)],
                    op0=OP.mult, op1=OP.add)
                pc = pc_full[:, GRP * g:GRP * (g + 1), :]
                nc.scalar.activation(
                    out=pc[:, :, 1], in_=dcol[:], func=AF.Sigmoid,
                    scale=it_s[:])
                # p0 = 1 - p1 (exact identity for sigmoid)
                nc.vector.tensor_scalar(
                    out=pc[:, :, 0], in0=pc[:, :, 1], scalar1=-1.0,
                    scalar2=1.0, op0=OP.mult, op1=OP.add)

                # EMA: group-batched matmuls (N=8), no serial dep
                cs = GRP * g
                if (cs % CH_ROW) == 0:
                    # chunks cc=0..3 of a row: chunk 0 uses A0 / feeds R*f
                    mms = [("a0t", cs, 1, 0, True),
                           ("amt", cs + 1, 3, 2, True),
                           ("r1f", cs, 1, 2, False),
                           ("r1m", cs + 1, 2, 4, False),
                           ("r2f", cs, 1, 4, False),
                           ("r2m", cs + 1, 1, 6, False)]
                else:
                    mms = [("amt", cs, 4, 0, True),
                           ("r1m", cs - 1, 4, 0, False),
                           ("r2m", cs - 2, 4, 0, False)]
                pst = ps.tile([128, 2 * GRP], f32, tag="s")
                for i, (mat, c0, n, off, st) in enumerate(mms):
                    nc.tensor.matmul(
                        pst[:, off:off + 2 * n], ema_s[mat][:],
                        pc_full[:, c0:c0 + n, :],
                        start=st, stop=(i == len(mms) - 1),
                        skip_group_check=True)
                nc.vector.tensor_copy(
                    out=s_all[:, cs:cs + GRP, :],
                    in_=pst[:].rearrange("p (c n) -> p c n", n=2))
                if g % 2 == 1:
                    r = g // 2
                    nc.sync.dma_start(
                        out=out_d[r].rearrange("(c p) n -> p c n", p=128),
                        in_=s_all[:, CH_ROW * r:CH_ROW * (r + 1), :])

            # chunk-granular software pipeline: stage offsets keep every
            # engine's in-order stream dense instead of draining group by
            # group at the end.
            D2A, D2B, DHE = 6, 11, 14
            NG = CH // GRP
            s1_chunk(0)
            load_rest()
            for t in range(1, CH + DHE + 1):
                if t < CH:
                    s1_chunk(t)
                if 0 <= t - D2A < CH:
                    s2a_chunk(t - D2A)
                if 0 <= t - D2B < CH:
                    s2b_chunk(t - D2B)
                if t >= DHE and (t - DHE) % GRP == 0 and (t - DHE) // GRP < NG:
                    head_ema((t - DHE) // GRP)

    if not sim_gelu:
        nc.compile()   # bacc pass pipeline (regalloc, wait splitting, ...)
    return nc


def _get_nc(triv1=True, triv2=True, trivb3=True):
    key = (triv1, triv2, trivb3)
    if key not in _NC:
        _NC[key] = _build_nc(triv1=triv1, triv2=triv2, trivb3=trivb3)
    return _NC[key]


def _host_inputs(inputs):
    """Build the per-core input maps from the full problem inputs."""
    x = np.ascontiguousarray(np.asarray(inputs["action_tokens"], np.float32))
    labels = np.asarray(inputs["critical_labels"]).astype(np.int32)
    W1 = np.asarray(inputs["W1"], np.float32)
    W2 = np.asarray(inputs["W2"], np.float32)
    W3 = np.asarray(inputs["W3"], np.float32)
    b1 = np.asarray(inputs["b1"], np.float32)
    b2 = np.asarray(inputs["b2"], np.float32)
    b3 = np.asarray(inputs["b3"], np.float32)
    g1 = np.asarray(inputs["g1"], np.float32)
    be1 = np.asarray(inputs["be1"], np.float32)
    g2 = np.asarray(inputs["g2"], np.float32)
    be2 = np.asarray(inputs["be2"], np.float32)
    temp = float(np.asarray(inputs["temperature"]))

    inv_t = np.float32(1.0 / max(temp, 0.1))
    ema = _make_ema_mats()

    w1p = np.ascontiguousarray(
        W1.reshape(KC, 128, HID1).transpose(1, 0, 2)).astype(_BF16)
    w2p = np.ascontiguousarray(
        W2.reshape(2, 128, HID2).transpose(1, 0, 2)).astype(_BF16)
    # h2g carries a factor 2 (erf-gelu without the 0.5) -> fold into W3
    w3p = (0.5 * W3).astype(_BF16)
    # h1g carries a factor 2 -> h2 = h1g'@W2 + 2*b2, LN2 eps scaled 4x
    b2p = (2.0 * b2).reshape(1, HID2).astype(_BF16)

    shared = {
        "w1": w1p,
        "w2": w2p,
        "w3": w3p,
        "b1": b1.reshape(1, HID1).astype(_BF16),
        "b2": b2p,
        "b3g": np.broadcast_to(np.tile(b3, GRP), (128, 2 * GRP))
                .astype(np.float32).copy(),
        # negated gains: the device-side rstd is negative (see rsqrt_full)
        "g1bn": np.broadcast_to(-g1, (128, HID1)).copy(),
        "be1b": np.broadcast_to(be1, (128, HID1)).copy(),
        "g2bn": np.broadcast_to(-g2, (128, HID2)).copy(),
        "be2b": np.broadcast_to(be2, (128, HID2)).copy(),
        **ema,
        "idbf": np.eye(128, dtype=_BF16),
        "idf32": np.eye(16, dtype=np.float32),
        "ones1": np.ones((1, 128), dtype=_BF16),
        "magici": np.full((128, 1), MAGIC, np.int32),
        "itb": np.full((128, 1), inv_t, np.float32),
        "nitb": np.full((128, 1), -inv_t, np.float32),
    }

    in_maps = []
    for core in range(NCORES):
        r0 = core * B_LOC
        m = dict(shared)
        m["x"] = np.ascontiguousarray(x[r0:r0 + B_LOC])
        m["labels"] = np.ascontiguousarray(
            labels[r0:r0 + B_LOC].reshape(CH, 128))
        in_maps.append(m)
    return in_maps


def kernel(**inputs) -> np.ndarray:
    global LAST_RESULTS
    from concourse.bass_utils import run_bass_kernel_spmd

    triv1 = (not np.any(np.asarray(inputs["b1"]))
             and np.all(np.asarray(inputs["g1"]) == 1)
             and not np.any(np.asarray(inputs["be1"])))
    triv2 = (not np.any(np.asarray(inputs["b2"]))
             and np.all(np.asarray(inputs["g2"]) == 1)
             and not np.any(np.asarray(inputs["be2"])))
    trivb3 = not np.any(np.asarray(inputs["b3"]))
    nc = _get_nc(triv1, triv2, trivb3)
    in_maps = _host_inputs(inputs)
    trace = bool(int(os.environ.get("BLSR_TRACE", "0")))
    res = run_bass_kernel_spmd(
        nc, in_maps, list(range(NCORES)), trace=trace)
    LAST_RESULTS = res
    out = np.concatenate([res.results[i]["out"] for i in range(NCORES)],
                         axis=0)
    return out.astype(np.float32)


# revision 72
# speedup vs baseline: 1.0152x; 1.0152x over previous
"""Trainium2 Bass kernel for nn_BinaryLabelSoftRouter.

Reference computation (B=16, T=1024, D=2048, H=256, H2=128):
  base   = where(labels>0, [.25,.75], [.75,.25])            # (B,T,2)
  h1     = gelu(LN(x @ W1 + b1) * g1 + be1)                 # erf gelu
  h2     = gelu(LN(h1 @ W2 + b2) * g2 + be2)
  adj    = tanh(h2 @ W3 + b3) * 0.1
  p      = softmax((base + adj) / clip(temp, .1), -1)       # (B,T,2)
  out    = EMA over T (s_t = .9 s_{t-1} + .1 p_t, s_0 = p_0)

Sharding: data-parallel over batch, 2 rows per core x 8 cores.

Device-side rewrites (all exact up to fp rounding):
  * softmax over 2 classes -> sigmoid of the logit difference.
  * EMA over each 128-step chunk is a lower-triangular [128,128] matmul
    plus a carry matmul from the previous chunk's last element.
  * gelu via erf:  2*gelu(x) = x*(1+erf(x/sqrt(2))).  The factor 2 on
    h1g cancels inside LN2 when LN2's eps is scaled 4x; the factor 2 on
    h2g is folded into W3 (host-side W3/2).  This keeps the scalar
    engine inside ONE activation-table set (sigmoid_and_others: copy /
    erf / sigmoid / tanh) -- act-table swaps cost ~1.3us each.
  * rstd = 1/sqrt(var+eps) via fast-inverse-sqrt (magic constant + 2
    Newton steps) on the vector engine, batched over 4 chunks, because
    Sqrt lives in a different act-table set.  The Newton iteration is
    signed so the final rstd comes out negative; the host passes -g1/-g2
    so the product is exact.

Main matmuls run in bf16 (fp32 PSUM accumulation) -> end-to-end rel
error vs the fp32 reference ~1e-4.  X is cast fp32->bf16 during the
HBM->SBUF DMA (SWDGE cast) and transposed on the tensor engine.
EMA matmuls run in fp32.
"""

import os
import numpy as np
import ml_dtypes

B, T, AD = 16, 1024, 2048
HID1, HID2 = 256, 128
NCORES = 8
B_LOC = B // NCORES            # 2 rows per core
CH_ROW = T // 128              # 8 chunks per row
CH = B_LOC * CH_ROW            # 16 chunks per core
GRP = 4                        # chunks per LN/head batch group
KC = AD // 128                 # 16 contraction chunks for mm1
SM = 0.9
ADJ = 0.1
LN_EPS = 1e-5
MAGIC = 0x5f3759df - 0x00400000   # seed for rsqrt of v2 = v/2

_BF16 = ml_dtypes.bfloat16

_NC = {}
LAST_RESULTS = None


def _make_ema_mats():
    """EMA-as-matmul constants, all pre-transposed to lhsT layout [k, tau].

    s_c = A_loc @ p_c + 0.9^(tau+1) * s_{c-1}[127] and the carry expands
    into rank-1 matmuls against p_{c-1}, p_{c-2}: contributions beyond
    depth 2 carry a 0.9^256 ~ 1.8e-12 factor -> exactly zero in fp32.
    This removes the serial cross-chunk dependency entirely.
    """
    tau = np.arange(128, dtype=np.float64)
    diff = tau[:, None] - tau[None, :]
    Am = np.where(diff >= 0, 0.1 * SM ** diff, 0.0)
    A0 = Am.copy()
    A0[:, 0] = SM ** tau
    dec = SM ** (tau + 1.0)          # 0.9^(tau+1)
    r1f = np.outer(A0[127, :], dec)  # [k, tau], carry from chunk 0
    r1m = np.outer(Am[127, :], dec)
    r2f = (SM ** 128) * r1f
    r2m = (SM ** 128) * r1m
    f32c = lambda a: np.ascontiguousarray(a, np.float32)
    return {
        "a0t": f32c(A0.T), "amt": f32c(Am.T),
        "r1f": f32c(r1f), "r1m": f32c(r1m),
        "r2f": f32c(r2f), "r2m": f32c(r2m),
    }


def _build_nc(sim_gelu=False, triv1=True, triv2=True, trivb3=True):
    # trivN: layer-N has b==0, g==1, be==0 (true for this problem's
    # setup_inputs); skips the bias matmul and the affine stt ops.
    # trivb3: b3 == 0.
    # sim_gelu: CoreSim has no Erf LUT; substitute Tanh so the identical
    # program structure can run under the simulator (race/OOB checks).
    import concourse.mybir as mybir
    import concourse.tile as tile
    from concourse import bacc

    f32 = mybir.dt.float32
    bf16 = mybir.dt.bfloat16
    i32 = mybir.dt.int32
    AF = mybir.ActivationFunctionType
    OP = mybir.AluOpType
    ERF = AF.Tanh if sim_gelu else AF.Erf
    INV_SQRT2 = float(1.0 / np.sqrt(2.0))

    nc = bacc.Bacc()

    # ---- DRAM parameters (per-core) ----
    x_d = nc.declare_dram_parameter("x", [B_LOC, T, AD], f32, isOutput=False)
    lab_d = nc.declare_dram_parameter("labels", [CH, 128], i32, isOutput=False)
    w1_d = nc.declare_dram_parameter("w1", [128, KC, HID1], bf16, isOutput=False)
    w2_d = nc.declare_dram_parameter("w2", [128, 2, HID2], bf16, isOutput=False)
    w3_d = nc.declare_dram_parameter("w3", [128, 2], bf16, isOutput=False)
    b1_d = nc.declare_dram_parameter("b1", [1, HID1], bf16, isOutput=False)
    b2_d = nc.declare_dram_parameter("b2", [1, HID2], bf16, isOutput=False)
    b3_d = nc.declare_dram_parameter("b3g", [128, 2 * GRP], f32, isOutput=False)
    g1_d = nc.declare_dram_parameter("g1bn", [128, HID1], f32, isOutput=False)
    be1_d = nc.declare_dram_parameter("be1b", [128, HID1], f32, isOutput=False)
    g2_d = nc.declare_dram_parameter("g2bn", [128, HID2], f32, isOutput=False)
    be2_d = nc.declare_dram_parameter("be2b", [128, HID2], f32, isOutput=False)
    ema_d = {
        name: nc.declare_dram_parameter(name, [128, 128], f32, isOutput=False)
        for name in ("a0t", "amt", "r1f", "r1m", "r2f", "r2m")
    }
    idb_d = nc.declare_dram_parameter("idbf", [128, 128], bf16, isOutput=False)
    idf_d = nc.declare_dram_parameter("idf32", [16, 16], f32, isOutput=False)
    ones_d = nc.declare_dram_parameter("ones1", [1, 128], bf16, isOutput=False)
    magic_d = nc.declare_dram_parameter("magici", [128, 1], i32, isOutput=False)
    it_d = nc.declare_dram_parameter("itb", [128, 1], f32, isOutput=False)
    nit_d = nc.declare_dram_parameter("nitb", [128, 1], f32, isOutput=False)
    out_d = nc.declare_dram_parameter("out", [B_LOC, T, 2], f32, isOutput=True)

    with tile.TileContext(nc) as tc:
        with (
            tc.tile_pool(name="singles", bufs=1) as singles,
            tc.tile_pool(name="xpool", bufs=3) as xpool,
            tc.tile_pool(name="xtpool", bufs=2) as xtpool,
            tc.tile_pool(name="act", bufs=4) as act,
            tc.tile_pool(name="hbuf", bufs=10) as hbuf,
            tc.tile_pool(name="stat", bufs=4) as stat,
            tc.tile_pool(name="ptp", bufs=3, space="PSUM") as ptp,
            tc.tile_pool(name="ptph", bufs=1, space="PSUM") as ptph,
            tc.tile_pool(name="pmm", bufs=2, space="PSUM") as pmm,
            tc.tile_pool(name="py", bufs=1, space="PSUM") as py,
            tc.tile_pool(name="ps", bufs=1, space="PSUM") as ps,
        ):
            # ---- resident tiles ----
            def load(name, shape, dt, src):
                t = singles.tile(shape, dt, tag=name)
                nc.sync.dma_start(t[:], src[:])
                return t

            # critical-path loads only; the rest is deferred until after
            # the first group's front end is emitted, so the first chunk's
            # activations aren't queued behind ~2 MB of constants.
            # chunk 0 goes via HWDGE fp32 + on-chip cast: it skips the
            # gpsimd SWDGE preamble and heads the DMA queue, so the tensor
            # engine starts transposing several us earlier.
            xc0f = singles.tile([128, AD], f32, tag="xc0f")
            nc.sync.dma_start(xc0f[:], x_d[0, 0:128, :])
            idb_s = load("idb", [128, 128], bf16, idb_d)
            w1_s = load("w1", [128, KC, HID1], bf16, w1_d)
            ones_s = (None if (triv1 and triv2)
                      else load("ones", [1, 128], bf16, ones_d))
            b1_s = None if triv1 else load("b1", [1, HID1], bf16, b1_d)
            idf_s = load("idf", [16, 16], f32, idf_d)

            # label prep: Lh[tau, chunk] = labels - 0.5 (tiny; done first
            # so its PE transpose doesn't stall the stream mid-kernel)
            lab_i = singles.tile([CH, 128], i32)
            nc.sync.dma_start(lab_i[:], lab_d[:])
            lab_f = singles.tile([CH, 128], f32)
            nc.vector.tensor_copy(lab_f[:], lab_i[:])
            p_lab = py.tile([128, CH], f32, tag="y")
            nc.tensor.transpose(p_lab[:], lab_f[:], idf_s[:])
            lh_s = singles.tile([128, CH], f32)
            nc.vector.tensor_scalar(
                out=lh_s[:], in0=p_lab[:], scalar1=0.5, scalar2=None,
                op0=OP.subtract)

            def load_rest():
                nonlocal w2_s, w3_s, b2_s, b3g_s, g1_s, be1_s, g2_s, \
                    be2_s, ema_s, magic_s, it_s, nit_s
                w2_s = load("w2", [128, 2, HID2], bf16, w2_d)
                w3_s = load("w3", [128, 2], bf16, w3_d)
                b2_s = None if triv2 else load("b2", [1, HID2], bf16, b2_d)
                b3g_s = (None if trivb3
                         else load("b3g", [128, 2 * GRP], f32, b3_d))
                g1_s = be1_s = g2_s = be2_s = None
                if not triv1:
                    g1_s = load("g1", [128, HID1], f32, g1_d)  # holds -g1
                    be1_s = load("be1", [128, HID1], f32, be1_d)
                if not triv2:
                    g2_s = load("g2", [128, HID2], f32, g2_d)  # holds -g2
                    be2_s = load("be2", [128, HID2], f32, be2_d)
                ema_s = {name: load(name, [128, 128], f32, d)
                         for name, d in ema_d.items()}
                magic_s = load("magic", [128, 1], i32, magic_d)
                it_s = load("it", [128, 1], f32, it_d)
                nit_s = load("nit", [128, 1], f32, nit_d)

            w2_s = w3_s = b2_s = b3g_s = g1_s = be1_s = g2_s = be2_s = None
            ema_s = magic_s = it_s = nit_s = None

            s_all = singles.tile([128, CH, 2], f32)
            pc_full = singles.tile([128, CH, 2], f32)

            def rsqrt_full(var_ap, n, eps, tagsuf):
                """negative 1/sqrt(var+eps) batched over n columns (fast
                inverse sqrt + 2 Newton steps; the sign is folded into the
                negated gains -g1/-g2 on the host side)."""
                v2 = stat.tile([128, n], f32, tag="v2" + tagsuf)
                nc.vector.tensor_scalar(
                    out=v2[:], in0=var_ap, scalar1=0.5, scalar2=0.5 * eps,
                    op0=OP.mult, op1=OP.add)
                ib = stat.tile([128, n], i32, tag="ib" + tagsuf)
                nc.vector.tensor_scalar(
                    out=ib[:], in0=v2[:].bitcast(i32), scalar1=1,
                    scalar2=None, op0=OP.logical_shift_right)
                y = stat.tile([128, n], f32, tag="y" + tagsuf)
                nc.vector.tensor_tensor(
                    out=y[:].bitcast(i32),
                    in0=magic_s[:].to_broadcast((128, n)), in1=ib[:],
                    op=OP.subtract)          # y0 = +seed
                p = stat.tile([128, n], f32, tag="p" + tagsuf)
                # iter 1: y1 = y0*(1.5 - v2*y0^2)  -> computed as
                #   p = y0*y0; q = p*v2; y1 = (q - 1.5)*y0 * -1 folded:
                # keep standard signs: y1 = (1.5 - q)*y0 via two ops
                nc.vector.tensor_tensor(out=p[:], in0=y[:], in1=y[:],
                                        op=OP.mult)
                nc.vector.tensor_tensor(out=p[:], in0=p[:], in1=v2[:],
                                        op=OP.mult)
                # y1n = (p - 1.5) * y0   = -y1   (negative)
                nc.vector.scalar_tensor_tensor(
                    out=y[:], in0=p[:], scalar=1.5, in1=y[:],
                    op0=OP.subtract, op1=OP.mult)
                # iter 2 on negative y1n: y1n^2 = y1^2 (sign cancels)
                nc.vector.tensor_tensor(out=p[:], in0=y[:], in1=y[:],
                                        op=OP.mult)
                nc.vector.tensor_tensor(out=p[:], in0=p[:], in1=v2[:],
                                        op=OP.mult)
                # y2n = (1.5 - p) * y1n  (stays negative):
                #     = (p - 1.5) * (-y1n)... use (p-1.5)*y1n = +y2; we
                # want negative output, so: y2n = (p - 1.5) * y1n * ...
                # (p-1.5) < 0 and y1n < 0 -> product positive = +y2.
                # One more negate folds into -g as planned, so produce +y2
                # here and pass -g:  final = (x-mu)*(-g)*(+y2)... wrong
                # sign.  Instead produce -y2: (1.5-p)*y1n.  No reverse
                # subtract available, so negate p first into (1.5-p) via
                # scalar_tensor_tensor with scalar=-1:
                #   y2n = ((p * -1) + 1.5) * y1n
                nc.vector.tensor_scalar(
                    out=p[:], in0=p[:], scalar1=-1.0, scalar2=1.5,
                    op0=OP.mult, op1=OP.add)
                nc.vector.tensor_tensor(out=y[:], in0=p[:], in1=y[:],
                                        op=OP.mult)   # negative rstd
                return y

            mv1G, h1sD, rstd1G = {}, {}, {}
            mv2G, h2sD, rstd2G, yallG = {}, {}, {}, {}
            xcD = {}

            def s1_chunk(c):
                """load + transpose + mm1 + LN1 stats for one chunk."""
                g, j = divmod(c, GRP)
                if j == 0:
                    mv1G[g] = stat.tile([128, GRP, 2], f32, tag="mv1", name=f"mv1_{g}")
                mv1 = mv1G[g]
                r, cc = divmod(c, CH_ROW)

                xc = xpool.tile([128, AD], bf16, tag="xc")
                if c == 0:
                    nc.vector.tensor_copy(out=xc[:], in_=xc0f[:])
                else:
                    for hh in range(2):
                        nc.gpsimd.dma_start(
                            out=xc[:, hh * (AD // 2):(hh + 1) * (AD // 2)],
                            in_=x_d[r, 128 * cc:128 * (cc + 1),
                                    hh * (AD // 2):(hh + 1) * (AD // 2)])

                xt = xtpool.tile([128, KC, 128], bf16, tag="xt")
                for tg in range(4):
                    ptile = ptp.tile([128, 512], bf16, tag="tp")
                    for tj in range(4):
                        k = 4 * tg + tj
                        nc.tensor.transpose(
                            ptile[:, 128 * tj:128 * (tj + 1)],
                            xc[:, 128 * k:128 * (k + 1)],
                            idb_s[:])
                    if tg % 2 == 0:
                        nc.scalar.activation(
                            out=xt[:, 4 * tg:4 * (tg + 1), :],
                            in_=ptile[:], func=AF.Copy)
                    else:
                        nc.vector.tensor_copy(
                            out=xt[:, 4 * tg:4 * (tg + 1), :],
                            in_=ptile[:])

                ph1 = pmm.tile([128, HID1], f32, tag="mm")
                for k in range(KC):
                    nc.tensor.matmul(
                        ph1[:], xt[:, k, :], w1_s[:, k, :],
                        start=(k == 0), stop=(triv1 and k == KC - 1))
                if not triv1:
                    nc.tensor.matmul(
                        ph1[:], ones_s[:], b1_s[:], start=False, stop=True)

                st6 = stat.tile([128, 6], f32, tag="st6")
                nc.vector.bn_stats(st6[:], ph1[:])
                nc.vector.bn_aggr(mv1[:, j, :], st6[:])
                h1s = hbuf.tile([128, HID1], f32, tag="h1s")
                nc.scalar.activation(out=h1s[:], in_=ph1[:], func=AF.Copy)
                h1sD[c] = h1s

            def s2a_chunk(c):
                """LN1 apply -> mm2 -> LN2 stats for one chunk."""
                g, j = divmod(c, GRP)
                if j == 0:
                    rstd1G[g] = rsqrt_full(mv1G[g][:, :, 1], GRP, LN_EPS,
                                           "a")
                    mv2G[g] = stat.tile([128, GRP, 2], f32, tag="mv2", name=f"mv2_{g}")
                mv1, rstd1, mv2 = mv1G[g], rstd1G[g], mv2G[g]
                h1s = h1sD.pop(c)

                xn = act.tile([128, HID1], f32, tag="xn")
                if triv1:
                    # xn = (h1 - mu) * (-rstd) = -LN(h1): one 2x-mode
                    # tensor_scalar; the sign cancels in the odd-erf
                    # gelu identity below.
                    nc.vector.tensor_scalar(
                        out=xn[:], in0=h1s[:], scalar1=mv1[:, j, 0:1],
                        scalar2=rstd1[:, j:j + 1],
                        op0=OP.subtract, op1=OP.mult)
                    sgn = -1.0
                else:
                    nc.vector.scalar_tensor_tensor(
                        out=xn[:], in0=h1s[:], scalar=mv1[:, j, 0:1],
                        in1=g1_s[:], op0=OP.subtract, op1=OP.mult)
                    nc.vector.scalar_tensor_tensor(
                        out=xn[:], in0=xn[:], scalar=rstd1[:, j:j + 1],
                        in1=be1_s[:], op0=OP.mult, op1=OP.add)
                    sgn = 1.0
                ef = act.tile([128, HID1], f32, tag="ef")
                nc.scalar.activation(out=ef[:], in_=xn[:], func=ERF,
                                     scale=INV_SQRT2)
                h1g = act.tile([128, HID1], bf16, tag="h1g")
                # 2*gelu(z) = (erf(z/sqrt2) + sgn) * xn  with xn=sgn*z
                nc.vector.scalar_tensor_tensor(
                    out=h1g[:], in0=ef[:], scalar=sgn, in1=xn[:],
                    op0=OP.add, op1=OP.mult)

                pt1 = ptph.tile([128, 512], bf16, tag="tph")
                for k in range(2):
                    nc.tensor.transpose(
                        pt1[:, 128 * k:128 * (k + 1)],
                        h1g[:, 128 * k:128 * (k + 1)],
                        idb_s[:])
                h1t = act.tile([128, 2, 128], bf16, tag="h1t")
                nc.scalar.activation(
                    out=h1t[:], in_=pt1[:, :256], func=AF.Copy)

                ph2 = pmm.tile([128, HID1], f32, tag="mm")
                for k in range(2):
                    nc.tensor.matmul(
                        ph2[:, :HID2], h1t[:, k, :], w2_s[:, k, :],
                        start=(k == 0), stop=(triv2 and k == 1))
                if not triv2:
                    nc.tensor.matmul(
                        ph2[:, :HID2], ones_s[:], b2_s[:], start=False,
                        stop=True)

                st6b = stat.tile([128, 6], f32, tag="st6")
                nc.vector.bn_stats(st6b[:], ph2[:, :HID2])
                nc.vector.bn_aggr(mv2[:, j, :], st6b[:])
                h2s = hbuf.tile([128, HID2], f32, tag="h2s")
                nc.scalar.activation(out=h2s[:], in_=ph2[:, :HID2],
                                     func=AF.Copy)
                h2sD[c] = h2s

            def s2b_chunk(c):
                """LN2 apply -> mm3 -> y for one chunk."""
                g, j = divmod(c, GRP)
                if j == 0:
                    # LN2 eps is 4x because h1g carries the factor 2
                    rstd2G[g] = rsqrt_full(mv2G[g][:, :, 1], GRP,
                                           4.0 * LN_EPS, "b")
                    yallG[g] = stat.tile([128, GRP, 2], f32, tag="yall",
                                         name=f"yall_{g}")
                mv2, rstd2, y_all = mv2G[g], rstd2G[g], yallG[g]
                h2s = h2sD.pop(c)

                xn2 = act.tile([128, HID2], f32, tag="xn2")
                if triv2:
                    nc.vector.tensor_scalar(
                        out=xn2[:], in0=h2s[:], scalar1=mv2[:, j, 0:1],
                        scalar2=rstd2[:, j:j + 1],
                        op0=OP.subtract, op1=OP.mult)
                    sgn2 = -1.0
                else:
                    nc.vector.scalar_tensor_tensor(
                        out=xn2[:], in0=h2s[:], scalar=mv2[:, j, 0:1],
                        in1=g2_s[:], op0=OP.subtract, op1=OP.mult)
                    nc.vector.scalar_tensor_tensor(
                        out=xn2[:], in0=xn2[:], scalar=rstd2[:, j:j + 1],
                        in1=be2_s[:], op0=OP.mult, op1=OP.add)
                    sgn2 = 1.0
                ef2 = act.tile([128, HID2], f32, tag="ef2")
                nc.scalar.activation(out=ef2[:], in_=xn2[:], func=ERF,
                                     scale=INV_SQRT2)
                h2g = act.tile([128, HID2], bf16, tag="h2g")
                nc.vector.scalar_tensor_tensor(
                    out=h2g[:], in0=ef2[:], scalar=sgn2, in1=xn2[:],
                    op0=OP.add, op1=OP.mult)

                pt2 = ptph.tile([128, 512], bf16, tag="tph")
                nc.tensor.transpose(pt2[:, :128], h2g[:], idb_s[:])
                h2t = act.tile([128, 128], bf16, tag="h2t")
                nc.vector.tensor_copy(out=h2t[:], in_=pt2[:, :128])
                pyt = py.tile([128, CH], f32, tag="y")
                nc.tensor.matmul(pyt[:, :2], h2t[:], w3_s[:],
                                 start=True, stop=True)
                nc.vector.tensor_copy(out=y_all[:, j, :], in_=pyt[:, :2])

            def head_ema(g):
                """batched head + EMA matmuls for one group."""
                y_all = yallG.pop(g)
                if not trivb3:
                    nc.vector.tensor_tensor(
                        out=y_all[:].rearrange("p g n -> p (g n)"),
                        in0=y_all[:].rearrange("p g n -> p (g n)"),
                        in1=b3g_s[:], op=OP.add)
                th = stat.tile([128, GRP, 2], f32, tag="th")
                nc.scalar.activation(
                    out=th[:].rearrange("p g n -> p (g n)"),
                    in_=y_all[:].rearrange("p g n -> p (g n)"),
                    func=AF.Tanh)
                dcol = stat.tile([128, GRP], f32, tag="dcol")
                nc.vector.tensor_tensor(
                    out=dcol[:], in0=th[:, :, 1], in1=th[:, :, 0],
                    op=OP.subtract)
                nc.vector.scalar_tensor_tensor(
                    out=dcol[:], in0=dcol[:], scalar=ADJ,
                    in1=lh_s[:, GRP * g:GRP * (g + 1)],
                    op0=OP.mult, op1=OP.add)
                pc = pc_full[:, GRP * g:GRP * (g + 1), :]
                nc.scalar.activation(
                    out=pc[:, :, 1], in_=dcol[:], func=AF.Sigmoid,
                    scale=it_s[:])
                # p0 = 1 - p1 (exact identity for sigmoid)
                nc.vector.tensor_scalar(
                    out=pc[:, :, 0], in0=pc[:, :, 1], scalar1=-1.0,
                    scalar2=1.0, op0=OP.mult, op1=OP.add)

                # EMA: group-batched matmuls (N=8), no serial dep
                cs = GRP * g
                if (cs % CH_ROW) == 0:
                    # chunks cc=0..3 of a row: chunk 0 uses A0 / feeds R*f
                    mms = [("a0t", cs, 1, 0, True),
                           ("amt", cs + 1, 3, 2, True),
                           ("r1f", cs, 1, 2, False),
                           ("r1m", cs + 1, 2, 4, False),
                           ("r2f", cs, 1, 4, False),
                           ("r2m", cs + 1, 1, 6, False)]
                else:
                    mms = [("amt", cs, 4, 0, True),
                           ("r1m", cs - 1, 4, 0, False),
                           ("r2m", cs - 2, 4, 0, False)]
                pst = ps.tile([128, 2 * GRP], f32, tag="s")
                for i, (mat, c0, n, off, st) in enumerate(mms):
                    nc.tensor.matmul(
                        pst[:, off:off + 2 * n], ema_s[mat][:],
                        pc_full[:, c0:c0 + n, :],
                        start=st, stop=(i == len(mms) - 1),
                        skip_group_check=True)
                nc.vector.tensor_copy(
                    out=s_all[:, cs:cs + GRP, :],
                    in_=pst[:].rearrange("p (c n) -> p c n", n=2))
                if g % 2 == 1:
                    r = g // 2
                    nc.sync.dma_start(
                        out=out_d[r].rearrange("(c p) n -> p c n", p=128),
                        in_=s_all[:, CH_ROW * r:CH_ROW * (r + 1), :])

            # chunk-granular software pipeline: stage offsets keep every
            # engine's in-order stream dense instead of draining group by
            # group at the end.
            D2A, D2B, DHE = 5, 10, 13
            NG = CH // GRP
            s1_chunk(0)
            load_rest()
            for t in range(1, CH + DHE + 1):
                if t < CH:
                    s1_chunk(t)
                if 0 <= t - D2A < CH:
                    s2a_chunk(t - D2A)
                if 0 <= t - D2B < CH:
                    s2b_chunk(t - D2B)
                if t >= DHE and (t - DHE) % GRP == 0 and (t - DHE) // GRP < NG:
                    head_ema((t - DHE) // GRP)

    if not sim_gelu:
        nc.compile()   # bacc pass pipeline (regalloc, wait splitting, ...)
    return nc


def _get_nc(triv1=True, triv2=True, trivb3=True):
    key = (triv1, triv2, trivb3)
    if key not in _NC:
        _NC[key] = _build_nc(triv1=triv1, triv2=triv2, trivb3=trivb3)
    return _NC[key]


def _host_inputs(inputs):
    """Build the per-core input maps from the full problem inputs."""
    x = np.ascontiguousarray(np.asarray(inputs["action_tokens"], np.float32))
    labels = np.asarray(inputs["critical_labels"]).astype(np.int32)
    W1 = np.asarray(inputs["W1"], np.float32)
    W2 = np.asarray(inputs["W2"], np.float32)
    W3 = np.asarray(inputs["W3"], np.float32)
    b1 = np.asarray(inputs["b1"], np.float32)
    b2 = np.asarray(inputs["b2"], np.float32)
    b3 = np.asarray(inputs["b3"], np.float32)
    g1 = np.asarray(inputs["g1"], np.float32)
    be1 = np.asarray(inputs["be1"], np.float32)
    g2 = np.asarray(inputs["g2"], np.float32)
    be2 = np.asarray(inputs["be2"], np.float32)
    temp = float(np.asarray(inputs["temperature"]))

    inv_t = np.float32(1.0 / max(temp, 0.1))
    ema = _make_ema_mats()

    w1p = np.ascontiguousarray(
        W1.reshape(KC, 128, HID1).transpose(1, 0, 2)).astype(_BF16)
    w2p = np.ascontiguousarray(
        W2.reshape(2, 128, HID2).transpose(1, 0, 2)).astype(_BF16)
    # h2g carries a factor 2 (erf-gelu without the 0.5) -> fold into W3
    w3p = (0.5 * W3).astype(_BF16)
    # h1g carries a factor 2 -> h2 = h1g'@W2 + 2*b2, LN2 eps scaled 4x
    b2p = (2.0 * b2).reshape(1, HID2).astype(_BF16)

    shared = {
        "w1": w1p,
        "w2": w2p,
        "w3": w3p,
        "b1": b1.reshape(1, HID1).astype(_BF16),
        "b2": b2p,
        "b3g": np.broadcast_to(np.tile(b3, GRP), (128, 2 * GRP))
                .astype(np.float32).copy(),
        # negated gains: the device-side rstd is negative (see rsqrt_full)
        "g1bn": np.broadcast_to(-g1, (128, HID1)).copy(),
        "be1b": np.broadcast_to(be1, (128, HID1)).copy(),
        "g2bn": np.broadcast_to(-g2, (128, HID2)).copy(),
        "be2b": np.broadcast_to(be2, (128, HID2)).copy(),
        **ema,
        "idbf": np.eye(128, dtype=_BF16),
        "idf32": np.eye(16, dtype=np.float32),
        "ones1": np.ones((1, 128), dtype=_BF16),
        "magici": np.full((128, 1), MAGIC, np.int32),
        "itb": np.full((128, 1), inv_t, np.float32),
        "nitb": np.full((128, 1), -inv_t, np.float32),
    }

    in_maps = []
    for core in range(NCORES):
        r0 = core * B_LOC
        m = dict(shared)
        m["x"] = np.ascontiguousarray(x[r0:r0 + B_LOC])
        m["labels"] = np.ascontiguousarray(
            labels[r0:r0 + B_LOC].reshape(CH, 128))
        in_maps.append(m)
    return in_maps


def kernel(**inputs) -> np.ndarray:
    global LAST_RESULTS
    from concourse.bass_utils import run_bass_kernel_spmd

    triv1 = (not np.any(np.asarray(inputs["b1"]))
             and np.all(np.asarray(inputs["g1"]) == 1)
             and not np.any(np.asarray(inputs["be1"])))
    triv2 = (not np.any(np.asarray(inputs["b2"]))
             and np.all(np.asarray(inputs["g2"]) == 1)
             and not np.any(np.asarray(inputs["be2"])))
    trivb3 = not np.any(np.asarray(inputs["b3"]))
    nc = _get_nc(triv1, triv2, trivb3)
    in_maps = _host_inputs(inputs)
    trace = bool(int(os.environ.get("BLSR_TRACE", "0")))
    res = run_bass_kernel_spmd(
        nc, in_maps, list(range(NCORES)), trace=trace)
    LAST_RESULTS = res
    out = np.concatenate([res.results[i]["out"] for i in range(NCORES)],
                         axis=0)
    return out.astype(np.float32)
